# revision 13
# baseline (speedup 1.0000x reference)
"""Trainium2 Bass kernel for a 2-layer GCN (nn_ClusterGNN):
    h1 = relu(gcn_conv(x, W1, b1)); out = log_softmax(gcn_conv(h1, W2, b2))

Strategy (8 NeuronCores, dest-node sharded):
  - The GCN symmetric norm factorizes: msg(r->c) = dinv[r]*dinv[c]*h[r].
    Tables are pre-scaled by dinv[src] at build time and the dinv[dst]
    factor is deferred into downstream per-partition scales, so edge
    aggregation is a pure unweighted segment-sum.
  - Segment-sum runs on the tensor engine: per 128-edge block, a one-hot
    selector S[e, d] = (col_local[e] == d) is built with one DVE
    tensor_scalar(is_equal) in fp16, then G^T[f, 128] += msg[128, F].T @
    S[128, 128] accumulates in PSUM fp32 per dest tile.
  - Edge messages come from raw InstDMAGatherAnt (fp16 rows, 128B for
    layer 1 / 64B for layer 2, 256B row stride) out of a per-core
    replicated table; int16 indices are window-local (32768-row windows).
  - Layer-2 table  dinv^2 * (relu(G1) @ W2)  is computed per dest shard
    compactly, AllGathered (6.4MB), and expanded into the strided table.
"""

import os
import sys

if "/opt/trn_rl_repo" not in sys.path:
    sys.path.insert(0, "/opt/trn_rl_repo")

import numpy as np

from concourse import bass, bacc, mybir, tile
from concourse.bass_utils import run_bass_kernel_spmd

P = 128
NCORES = 8
WIN = 32768
CHUNK_T = 16  # dest tiles per gather chunk
TROW = 128    # table row stride in fp16 elements (= 256B)

F32 = mybir.dt.float32
F16 = mybir.dt.float16
I16 = mybir.dt.int16


def cdiv(a, b):
    return -(-a // b)


class Cfg:
    pass


def raw_gather(nc, out_ap, in_ap, idxs_ap, num_idxs, elem_size):
    """InstDMAGatherAnt with arbitrary elem_size (bytes%256 need not hold);
    row stride fixed at 256B (stride_bytes_256=1)."""
    gp = nc.gpsimd
    _in_ap = gp.lower_ap_dma(in_ap, for_custom_bir_dma=True)
    _idxs_ap = gp.lower_ap(idxs_ap)
    _out_ap = gp.lower_ap(out_ap)
    return gp.add_instruction(
        mybir.InstDMAGatherAnt(
            name=nc.get_next_instruction_name(),
            ins=[*_in_ap, _idxs_ap, gp.lower_val_access(gp.to_reg(num_idxs))],
            outs=[_out_ap],
            transpose=False,
            num_idxs=num_idxs,
            elem_size=elem_size,
            stride_bytes_256=1,
            gen_mode=0,
            single_packet=False,
            queue_num=0,
            sbuf_tokens_per_rank=0,
            sbuf_free_dim_per_rank=0,
            sbuf_free_dim_pad_per_rank=0,
            sbuf_byte_offset=0,
        )
    )


def prep(x, edge_index, W1, b1, W2, b2):
    """Host-side layout prep (shard, sort, window-group, pad)."""
    x = np.asarray(x, dtype=np.float32)
    W1 = np.asarray(W1, dtype=np.float32)
    b1 = np.asarray(b1, dtype=np.float32)
    W2 = np.asarray(W2, dtype=np.float32)
    b2 = np.asarray(b2, dtype=np.float32)

    N, FIN = x.shape
    FH = W1.shape[1]
    FO = W2.shape[1]

    cfg = Cfg()
    cfg.N, cfg.FIN, cfg.FH, cfg.FO = N, FIN, FH, FO
    SHARD_T = cdiv(N, P * NCORES)
    NT = SHARD_T * NCORES
    PAD_N = NT * P
    SHARD_N = SHARD_T * P
    cfg.SHARD_T, cfg.NT, cfg.PAD_N, cfg.SHARD_N = SHARD_T, NT, PAD_N, SHARD_N
    NW = cdiv(PAD_N, WIN)
    cfg.NW = NW
    cfg.win_rows = [min(PAD_N, (w + 1) * WIN) - w * WIN for w in range(NW)]
    cfg.use_b1 = bool(np.any(b1))
    cfg.use_b2 = bool(np.any(b2))

    row = np.asarray(edge_index[0], dtype=np.int64)
    col = np.asarray(edge_index[1], dtype=np.int64)
    loops = np.arange(N, dtype=np.int64)
    src_all = np.concatenate([row, loops])
    dst_all = np.concatenate([col, loops])

    # sort edges by (dest tile, window, dst)
    tile_of = dst_all // P
    win_of = src_all // WIN
    key = (tile_of * NW + win_of) * np.int64(PAD_N) + dst_all
    order = np.argsort(key, kind="stable")
    src_s = src_all[order]
    dst_s = dst_all[order]
    tw_s = (tile_of * NW + win_of)[order]

    # counts per (global tile, window) -> shared nblk (max over cores)
    cnts = np.bincount(tw_s, minlength=NT * NW).reshape(NCORES, SHARD_T, NW)
    nblk = cdiv(cnts, P).max(axis=0)  # [SHARD_T, NW]
    cfg.nblk = nblk

    # chunks of dest tiles
    chunks = []
    j = 0
    while j < SHARD_T:
        chunks.append((j, min(j + CHUNK_T, SHARD_T)))
        j += CHUNK_T
    cfg.chunks = chunks

    # slot layout (shared): for chunk: for w: for j in chunk
    slot_start = np.zeros((SHARD_T, NW), dtype=np.int64)
    gathers = []  # per chunk: (w, slot0, nslots)
    blocks_of_tile = [[] for _ in range(SHARD_T)]
    chunk_slot0 = []
    s = 0
    for (j0, j1) in chunks:
        chunk_slot0.append(s)
        glist = []
        for w in range(NW):
            sw0 = s
            for j in range(j0, j1):
                slot_start[j, w] = s
                nb = int(nblk[j, w])
                blocks_of_tile[j].extend(range(s // P, s // P + nb))
                s += nb * P
            if s > sw0:
                glist.append((w, sw0, s - sw0))
        gathers.append(glist)
    SLOTS = s
    NB = SLOTS // P
    cfg.SLOTS, cfg.NB = SLOTS, NB
    cfg.gathers = gathers
    cfg.chunk_slot0 = chunk_slot0
    cfg.blocks_of_tile = blocks_of_tile

    # real-edge CSR (degree metadata)
    deg_edges = np.bincount(col, minlength=PAD_N).astype(np.int64)
    rp_edges = np.concatenate([[0], np.cumsum(deg_edges)])
    rpl = rp_edges[:-1].astype(np.float32)
    rph = rp_edges[1:].astype(np.float32)

    # x^T fp16, padded, pair-permuted columns
    xT = np.zeros((FIN, PAD_N), dtype=np.float16)
    xT[:, :N] = x.T.astype(np.float16)
    perm = (
        np.arange(PAD_N).reshape(-1, P, 2).transpose(0, 2, 1).reshape(-1)
    )
    xT_perm = xT[:, perm]
    pc = perm.reshape(NT, P).T
    rpl_p = rpl[pc]
    rph_p = rph[pc]

    iota = np.broadcast_to(np.arange(P, dtype=np.float16)[None, :], (P, P)).copy()
    ident = np.eye(FO, dtype=np.float32)
    W1p = np.zeros((FIN, TROW), dtype=np.float16)
    W1p[:, :FH] = W1.astype(np.float16)
    W2h = W2.astype(np.float16)  # [FH, FO]
    b2rep = np.broadcast_to(b2[None, :], (P, FO)).copy()
    b1r = b1[None, :].copy()

    rp_tiles = np.searchsorted(tw_s, np.arange(NT * NW + 1))

    in_maps = []
    for k in range(NCORES):
        idx_flat = np.zeros(SLOTS, dtype=np.int16)
        col_flat = np.full(SLOTS, -1.0, dtype=np.float32)
        for j in range(SHARD_T):
            t = k * SHARD_T + j
            for w in range(NW):
                a, b = rp_tiles[t * NW + w], rp_tiles[t * NW + w + 1]
                n = b - a
                if n == 0:
                    continue
                s0 = slot_start[j, w]
                idx_flat[s0:s0 + n] = (src_s[a:b] - w * WIN).astype(np.int16)
                col_flat[s0:s0 + n] = (dst_s[a:b] - t * P).astype(np.float32)
        idx16 = np.tile(idx_flat.reshape(-1, 16).T, (NCORES, 1))
        col_arr = col_flat.reshape(NB, P).T.copy()

        gn = (k * SHARD_N + np.arange(SHARD_N)).reshape(SHARD_T, P).T
        in_map = {
            "xT": xT_perm,
            "w1p": W1p,
            "w2h": W2h,
            "iota": iota,
            "ident": ident,
            "b2rep": b2rep,
            "rpl_p": rpl_p,
            "rph_p": rph_p,
            "rpl_g": rpl[gn].copy(),
            "rph_g": rph[gn].copy(),
            "idx16": idx16,
            "colv": col_arr,
        }
        if cfg.use_b1:
            rn = k * SHARD_N + np.arange(SHARD_N)
            in_map["b1r"] = b1r
            in_map["rpl_r"] = rpl[rn][None, :].copy()
            in_map["rph_r"] = rph[rn][None, :].copy()
        in_maps.append(in_map)

    return cfg, in_maps


def build_program(cfg):
    FIN, FH, FO = cfg.FIN, cfg.FH, cfg.FO
    NT, SHARD_T, PAD_N, SHARD_N = cfg.NT, cfg.SHARD_T, cfg.PAD_N, cfg.SHARD_N
    NW, NB, SLOTS = cfg.NW, cfg.NB, cfg.SLOTS

    nc = bacc.Bacc(
        "TRN2", target_bir_lowering=False, debug=False, num_devices=NCORES
    )

    xT_in = nc.dram_tensor("xT", [FIN, PAD_N], F16, kind="ExternalInput").ap()
    w1p_in = nc.dram_tensor("w1p", [FIN, TROW], F16, kind="ExternalInput").ap()
    w2h_in = nc.dram_tensor("w2h", [FH, FO], F16, kind="ExternalInput").ap()
    iota_in = nc.dram_tensor("iota", [P, P], F16, kind="ExternalInput").ap()
    ident_in = nc.dram_tensor("ident", [FO, FO], F32, kind="ExternalInput").ap()
    b2rep_in = nc.dram_tensor("b2rep", [P, FO], F32, kind="ExternalInput").ap()
    rpl_p_in = nc.dram_tensor("rpl_p", [P, NT], F32, kind="ExternalInput").ap()
    rph_p_in = nc.dram_tensor("rph_p", [P, NT], F32, kind="ExternalInput").ap()
    rpl_g_in = nc.dram_tensor("rpl_g", [P, SHARD_T], F32, kind="ExternalInput").ap()
    rph_g_in = nc.dram_tensor("rph_g", [P, SHARD_T], F32, kind="ExternalInput").ap()
    idx_in = nc.dram_tensor("idx16", [P, SLOTS // 16], I16, kind="ExternalInput").ap()
    col_in = nc.dram_tensor("colv", [P, NB], F32, kind="ExternalInput").ap()
    if cfg.use_b1:
        b1r_in = nc.dram_tensor("b1r", [1, FH], F32, kind="ExternalInput").ap()
        rpl_r_in = nc.dram_tensor("rpl_r", [1, SHARD_N], F32, kind="ExternalInput").ap()
        rph_r_in = nc.dram_tensor("rph_r", [1, SHARD_N], F32, kind="ExternalInput").ap()

    table1 = nc.dram_tensor("table1", [PAD_N, TROW], F16, kind="Internal").ap()
    t2c = nc.dram_tensor("t2c", [SHARD_N, FO], F16, kind="Internal").ap()
    t2full = nc.dram_tensor(
        "t2full", [PAD_N, FO], F16, kind="Internal", addr_space="Shared"
    ).ap()
    table2 = nc.dram_tensor("table2", [PAD_N, TROW], F16, kind="Internal").ap()
    out = nc.dram_tensor("out", [SHARD_N, FO], F32, kind="ExternalOutput").ap()

    stage = os.environ.get("BASSGNN_STAGE", "full")

    with tile.TileContext(nc) as tc:
        with tc.tile_pool(name="const", bufs=1) as cpool:
            w1_t = cpool.tile([FIN, TROW], F16)
            nc.sync.dma_start(out=w1_t[:], in_=w1p_in[:, :])
            w2_t = cpool.tile([FH, FO], F16)
            nc.sync.dma_start(out=w2_t[:], in_=w2h_in[:, :])
            iota_t = cpool.tile([P, P], F16)
            nc.sync.dma_start(out=iota_t[:], in_=iota_in[:, :])
            ident_t = cpool.tile([FO, FO], F32)
            nc.sync.dma_start(out=ident_t[:], in_=ident_in[:, :])
            b2_t = cpool.tile([P, FO], F32)
            nc.sync.dma_start(out=b2_t[:], in_=b2rep_in[:, :])
            col_t = cpool.tile([P, NB], F32)
            nc.sync.dma_start(out=col_t[:], in_=col_in[:, :])

            def make_dinv(lo_in, hi_in, n, tag):
                lo = cpool.tile([P, n], F32, tag=f"{tag}_lo")
                nc.sync.dma_start(out=lo[:], in_=lo_in[:, :])
                hi = cpool.tile([P, n], F32, tag=f"{tag}_hi")
                nc.sync.dma_start(out=hi[:], in_=hi_in[:, :])
                d = cpool.tile([P, n], F32, tag=f"{tag}_d")
                nc.vector.tensor_tensor(
                    out=d[:], in0=hi[:], in1=lo[:], op=mybir.AluOpType.subtract
                )
                nc.scalar.activation(
                    out=d[:], in_=d[:],
                    func=mybir.ActivationFunctionType.Sqrt, bias=1.0,
                )
                nc.vector.reciprocal(out=d[:], in_=d[:])
                return d

            dinv_p = make_dinv(rpl_p_in, rph_p_in, NT, "dp")
            dinv_g = make_dinv(rpl_g_in, rph_g_in, SHARD_T, "dg")
            dinv_gsq = cpool.tile([P, SHARD_T], F32)
            nc.vector.tensor_tensor(
                out=dinv_gsq[:], in0=dinv_g[:], in1=dinv_g[:],
                op=mybir.AluOpType.mult,
            )
            if cfg.use_b1:
                b1_t = cpool.tile([1, FH], F32)
                nc.sync.dma_start(out=b1_t[:], in_=b1r_in[:, :])
                lo = cpool.tile([1, SHARD_N], F32, tag="sq_lo")
                nc.sync.dma_start(out=lo[:], in_=rpl_r_in[:, :])
                hi = cpool.tile([1, SHARD_N], F32, tag="sq_hi")
                nc.sync.dma_start(out=hi[:], in_=rph_r_in[:, :])
                sqd_t = cpool.tile([1, SHARD_N], F32)
                nc.vector.tensor_tensor(
                    out=sqd_t[:], in0=hi[:], in1=lo[:],
                    op=mybir.AluOpType.subtract,
                )
                nc.scalar.activation(
                    out=sqd_t[:], in_=sqd_t[:],
                    func=mybir.ActivationFunctionType.Sqrt, bias=1.0,
                )

            # ---- Phase B: table1 = dinv * (x @ W1), full table per core ----
            with (
                tc.tile_pool(name="phb", bufs=4) as bpool,
                tc.tile_pool(name="phb_st", bufs=3) as stpool,
                tc.tile_pool(name="phb_ps", bufs=4, space="PSUM") as bpsum,
            ):
                GT = 16 if NT % 16 == 0 else 8
                assert NT % GT == 0
                for g8 in range(NT // GT):
                    xt8 = bpool.tile([P, GT * P], F16, tag="xt")
                    nc.sync.dma_start(
                        out=xt8[:], in_=xT_in[:, g8 * GT * P:(g8 + 1) * GT * P]
                    )
                    st8 = stpool.tile([P, GT * TROW], F16, tag="st")
                    for i in range(GT):
                        t = g8 * GT + i
                        hp = bpsum.tile([P, TROW], F32, tag="hp")
                        nc.tensor.matmul(
                            out=hp[:], lhsT=xt8[:, i * P:(i + 1) * P],
                            rhs=w1_t[:], start=True, stop=True,
                        )
                        q, two = i // 2, i % 2
                        nc.scalar.activation(
                            out=st8[:, (q * 2 + two) * TROW:(q * 2 + two + 1) * TROW],
                            in_=hp[:],
                            func=mybir.ActivationFunctionType.Copy,
                            scale=dinv_p[:, t:t + 1],
                        )
                    nc.sync.dma_start(
                        out=table1[
                            g8 * GT * P:(g8 + 1) * GT * P, :
                        ].rearrange("(q p two) f -> p q (two f)", two=2, p=P),
                        in_=st8[:].rearrange("p (q f) -> p q f", q=GT // 2),
                    )

            tc.strict_bb_all_engine_barrier()

            # ---- aggregation over edges ----
            def agg_layer(layer):
                tbl = table1 if layer == 1 else table2
                FA = FH if layer == 1 else FO
                with (
                    tc.tile_pool(name=f"msg{layer}", bufs=2) as mpool,
                    tc.tile_pool(name=f"idx{layer}", bufs=2) as ipool,
                    tc.tile_pool(name=f"s{layer}", bufs=4) as spool,
                    tc.tile_pool(name=f"work{layer}", bufs=3) as wpool,
                    tc.tile_pool(name=f"ps{layer}", bufs=4, space="PSUM") as ppool,
                    tc.tile_pool(name=f"ps{layer}b", bufs=2, space="PSUM") as qpool,
                ):
                    for ci, (j0, j1) in enumerate(cfg.chunks):
                        sc0 = cfg.chunk_slot0[ci]
                        cslots = sum(
                            int(cfg.nblk[j, w]) * P
                            for j in range(j0, j1) for w in range(NW)
                        )
                        ckb = cslots // P
                        msg = mpool.tile([P, ckb * FA], F16, tag="msg")
                        idxt = ipool.tile([P, cslots // 16], I16, tag="idx")
                        nc.sync.dma_start(
                            out=idxt[:],
                            in_=idx_in[:, sc0 // 16: sc0 // 16 + cslots // 16],
                        )
                        if layer == 1:
                            st2big = wpool.tile([P, (j1 - j0) * FO], F16, tag="st2b")
                        else:
                            otbig = wpool.tile([P, (j1 - j0) * FO], F32, tag="otb")
                        for (w, sw0, nw_slots) in cfg.gathers[ci]:
                            bw0 = (sw0 - sc0) // P
                            nbw = nw_slots // P
                            raw_gather(
                                nc,
                                out_ap=msg[
                                    :, bw0 * FA:(bw0 + nbw) * FA
                                ].rearrange("p (b f) -> p b f", f=FA),
                                in_ap=tbl[
                                    w * WIN: w * WIN + cfg.win_rows[w], :FA
                                ],
                                idxs_ap=idxt[
                                    :, (sw0 - sc0) // 16:
                                    (sw0 - sc0 + nw_slots) // 16
                                ],
                                num_idxs=nw_slots,
                                elem_size=FA,
                            )
                        for j in range(j0, j1):
                            blocks = cfg.blocks_of_tile[j]
                            nb = len(blocks)
                            gt = ppool.tile([FA, P], F32, tag="gt")
                            for i, b in enumerate(blocks):
                                bl = b - sc0 // P
                                s_t = spool.tile([P, P], F16, tag="s")
                                nc.vector.tensor_scalar(
                                    out=s_t[:],
                                    in0=iota_t[:],
                                    scalar1=col_t[:, b:b + 1],
                                    scalar2=None,
                                    op0=mybir.AluOpType.is_equal,
                                )
                                nc.tensor.matmul(
                                    out=gt[:],
                                    lhsT=msg[:, bl * FA:(bl + 1) * FA],
                                    rhs=s_t[:],
                                    start=(i == 0),
                                    stop=(
                                        i == nb - 1
                                        and not (layer == 1 and cfg.use_b1)
                                    ),
                                )
                            ti = j - j0
                            if layer == 1:
                                if cfg.use_b1:
                                    nc.tensor.matmul(
                                        out=gt[:],
                                        lhsT=b1_t[:, :],
                                        rhs=sqd_t[:, j * P:(j + 1) * P],
                                        start=False,
                                        stop=True,
                                    )
                                r1 = wpool.tile([FH, P], F16, tag="r1")
                                nc.scalar.activation(
                                    out=r1[:], in_=gt[:],
                                    func=mybir.ActivationFunctionType.Relu,
                                )
                                h2p = qpool.tile([P, FO], F32, tag="h2")
                                nc.tensor.matmul(
                                    out=h2p[:], lhsT=r1[:], rhs=w2_t[:],
                                    start=True, stop=True,
                                )
                                nc.scalar.activation(
                                    out=st2big[:, ti * FO:(ti + 1) * FO],
                                    in_=h2p[:],
                                    func=mybir.ActivationFunctionType.Copy,
                                    scale=dinv_gsq[:, j:j + 1],
                                )
                            else:
                                g2s = wpool.tile([FO, P], F32, tag="g2s")
                                nc.vector.tensor_copy(out=g2s[:], in_=gt[:])
                                op = qpool.tile([P, FO], F32, tag="op")
                                nc.tensor.transpose(
                                    out=op[:], in_=g2s[:], identity=ident_t[:]
                                )
                                o1 = wpool.tile([P, FO], F32, tag="o1")
                                nc.scalar.activation(
                                    out=o1[:], in_=op[:],
                                    func=mybir.ActivationFunctionType.Copy,
                                    scale=dinv_g[:, j:j + 1],
                                )
                                if cfg.use_b2:
                                    nc.vector.tensor_tensor(
                                        out=o1[:], in0=o1[:], in1=b2_t[:],
                                        op=mybir.AluOpType.add,
                                    )
                                nm = wpool.tile([P, 1], F32, tag="nm")
                                nc.vector.tensor_reduce(
                                    out=nm[:], in_=o1[:],
                                    axis=mybir.AxisListType.X,
                                    op=mybir.AluOpType.max, negate=True,
                                )
                                e_t = wpool.tile([P, FO], F32, tag="e")
                                nc.scalar.activation(
                                    out=e_t[:], in_=o1[:],
                                    func=mybir.ActivationFunctionType.Exp,
                                    bias=nm[:, :1],
                                )
                                ssum = wpool.tile([P, 1], F32, tag="ss")
                                nc.vector.tensor_reduce(
                                    out=ssum[:], in_=e_t[:],
                                    axis=mybir.AxisListType.X,
                                    op=mybir.AluOpType.add,
                                )
                                ls = wpool.tile([P, 1], F32, tag="ls")
                                nc.scalar.activation(
                                    out=ls[:], in_=ssum[:],
                                    func=mybir.ActivationFunctionType.Ln,
                                )
                                mls = wpool.tile([P, 1], F32, tag="mls")
                                nc.vector.tensor_tensor(
                                    out=mls[:], in0=ls[:], in1=nm[:],
                                    op=mybir.AluOpType.subtract,
                                )
                                nc.vector.tensor_scalar(
                                    out=otbig[:, ti * FO:(ti + 1) * FO],
                                    in0=o1[:],
                                    scalar1=mls[:, :1], scalar2=None,
                                    op0=mybir.AluOpType.subtract,
                                )
                        if layer == 1:
                            nc.sync.dma_start(
                                out=t2c[
                                    j0 * P:j1 * P, :
                                ].rearrange("(t p) f -> p t f", p=P),
                                in_=st2big[:].rearrange(
                                    "p (t f) -> p t f", t=j1 - j0
                                ),
                            )
                        else:
                            nc.sync.dma_start(
                                out=out[
                                    j0 * P:j1 * P, :
                                ].rearrange("(t p) f -> p t f", p=P),
                                in_=otbig[:].rearrange(
                                    "p (t f) -> p t f", t=j1 - j0
                                ),
                            )

            if stage != "b":
                agg_layer(1)

            if stage in ("full", "nocoll"):
                tc.strict_bb_all_engine_barrier()
                if stage == "full":
                    nc.gpsimd.collective_compute(
                        "AllGather",
                        mybir.AluOpType.bypass,
                        replica_groups=[list(range(NCORES))],
                        ins=[t2c[:, :]],
                        outs=[t2full[:, :]],
                    )
                # expand compact [PAD_N, FO] into strided table2[:, :FO]
                src_t = t2full if stage == "full" else t2c
                nrow = PAD_N if stage == "full" else SHARD_N
                with tc.tile_pool(name="expand", bufs=3) as epool:
                    ET = 64  # tiles per expand group
                    for g in range(0, nrow // P, ET):
                        ge = min(g + ET, nrow // P)
                        ex = epool.tile([P, (ge - g) * FO], F16, tag="ex")
                        nc.sync.dma_start(
                            out=ex[:],
                            in_=src_t[g * P:ge * P, :].rearrange(
                                "(t p) f -> p t f", p=P
                            ),
                        )
                        nc.sync.dma_start(
                            out=table2[g * P:ge * P, :FO].rearrange(
                                "(t p) f -> p t f", p=P
                            ),
                            in_=ex[:].rearrange("p (t f) -> p t f", t=ge - g),
                        )
                tc.strict_bb_all_engine_barrier()
                agg_layer(2)

    nc.compile()
    return nc


_CACHE = {}
TRACE = False
LAST = None


def kernel(x, edge_index, W1, b1, W2, b2):
    global LAST
    x = np.asarray(x)
    N = x.shape[0]
    cfg, in_maps = prep(x, edge_index, W1, b1, W2, b2)
    key = (
        N, cfg.FIN, cfg.FH, cfg.FO, cfg.SLOTS, cfg.use_b1, cfg.use_b2,
        tuple(int(v) for v in cfg.K_t.tolist())
        if hasattr(cfg, "K_t") else tuple(cfg.nblk.reshape(-1).tolist()),
    )
    if key not in _CACHE:
        _CACHE[key] = build_program(cfg)
    nc = _CACHE[key]
    try:
        res = run_bass_kernel_spmd(
            nc, in_maps, core_ids=list(range(NCORES)), trace=TRACE
        )
    except Exception:
        # transient device wedge (NRT_EXEC_UNIT_UNRECOVERABLE) -- retry once
        import time as _time
        _time.sleep(10)
        res = run_bass_kernel_spmd(
            nc, in_maps, core_ids=list(range(NCORES)), trace=TRACE
        )
    LAST = res
    outs = [res.results[k]["out"] for k in range(NCORES)]
    full = np.concatenate(outs, axis=0)[:N]
    return full.astype(np.float32)
